# revision 1
# baseline (speedup 1.0000x reference)
"""Multi-head attention (B=4, S=2048, D=1024, H=16, Dh=64) on 8 NeuronCores.

Sharding: core c handles batch b=c//2 and head-group g=c%2 (8 heads).
wq/wk/wv column-parallel, wo row-parallel; host sums the two partial
wo-products per batch and adds bo.

Per-core kernel (matmul operands in bf16 = 1 cyc/row PE streaming + FWL;
accumulation always f32 in PSUM; softmax normalization in f32):
  phase 1: Q^T,K^T [512,2048] and V [2048, 8x(64+ones)] projections
  phase 2: causal flash attention per (head, 512-wide q chunk):
           scores_T[sk,sq] = K^T_tile.T @ Q^T_chunk  (K=64 contraction)
           attn_T = exp(0.125*scores + causal_mask)   (ACT, bf16 out)
           outT_aug[65,sq] += [v|1].T @ attn_T        (PSUM accumulate)
           normalize: bcast denom row via PE outer product, then
           reciprocal_approx_fast + multiply in f32 on PSUM
  phase 3: out_partial[s,1024] = attn_outT.T @ woT   (K=512 per core)
"""

import sys

sys.path.insert(0, "/opt/trn_rl_repo")

import ml_dtypes
import numpy as np

import concourse.bass as bass  # noqa: F401
import concourse.bacc as bacc
import concourse.tile as tile
import concourse.mybir as mybir
from concourse.bass_utils import run_bass_kernel_spmd

F32 = mybir.dt.float32
F32R = mybir.dt.float32r
BF16 = mybir.dt.bfloat16
AF = mybir.ActivationFunctionType
BF = ml_dtypes.bfloat16

B, S, D = 4, 2048, 1024
H, DH = 16, 64
HG = 8  # heads per core
DG = HG * DH  # 512 out-dims per core
NEG = -1.0e9

_PROGRAM = None
LAST_RESULTS = None  # for test.py introspection


def _build_program():
    nc = bacc.Bacc("TRN2", target_bir_lowering=False, debug=False)

    xq_t = nc.dram_tensor("xq_t", [D, S], BF16, kind="ExternalInput")
    xk_t = nc.dram_tensor("xk_t", [D, S], BF16, kind="ExternalInput")
    xv_t = nc.dram_tensor("xv_t", [D, S], BF16, kind="ExternalInput")
    wq_t = nc.dram_tensor("wq_t", [D, DG], BF16, kind="ExternalInput")
    wk_t = nc.dram_tensor("wk_t", [D, DG], BF16, kind="ExternalInput")
    wv_t = nc.dram_tensor("wv_t", [D, DG], BF16, kind="ExternalInput")
    wo_t = nc.dram_tensor("wo_t", [DG, D], BF16, kind="ExternalInput")
    bq_c = nc.dram_tensor("bq_c", [128, 4], F32, kind="ExternalInput")
    bk_c = nc.dram_tensor("bk_c", [128, 4], F32, kind="ExternalInput")
    bv_r = nc.dram_tensor("bv_r", [1, DG], BF16, kind="ExternalInput")
    ones_b = nc.dram_tensor("ones_b", [1, 128], BF16, kind="ExternalInput")
    ones_f = nc.dram_tensor("ones_f", [1, 128], F32R, kind="ExternalInput")
    ones8 = nc.dram_tensor("ones8", [128, 8], BF16, kind="ExternalInput")
    maskadd = nc.dram_tensor("maskadd", [128, 128], F32, kind="ExternalInput")
    mask01 = nc.dram_tensor("mask01", [128, 128], BF16, kind="ExternalInput")
    outs = [
        nc.dram_tensor(f"out{c}", [S, D], F32, kind="ExternalOutput")
        for c in range(4)
    ]

    with tile.TileContext(nc) as tc:
        with (
            nc.allow_low_precision(reason="bf16 attention pipeline"),
            tc.tile_pool(name="persist", bufs=1) as pers,
        ):
            # ---- persistent tiles ----
            qT = [pers.tile([128, S], BF16, name=f"qT{i}") for i in range(4)]
            kT = [pers.tile([128, S], BF16, name=f"kT{i}") for i in range(4)]
            # v tiles: [128 s, 8 heads x (64 v + 1 ones)]
            vt = [pers.tile([128, HG * 65], BF16, name=f"v{i}") for i in range(16)]
            aout = [pers.tile([128, S], BF16, name=f"ao{i}") for i in range(4)]
            mask_sb = pers.tile([128, 128], F32, name="mask")
            mask01_sb = pers.tile([128, 128], BF16, name="mask01")
            ones_bf = pers.tile([1, 128], BF16, name="ones_bf")
            ones_fr = pers.tile([65, 128], F32R, name="ones_fr")
            bq_sb = pers.tile([128, 4], F32, name="bq")
            bk_sb = pers.tile([128, 4], F32, name="bk")
            bv_sb = pers.tile([1, DG], BF16, name="bv")

            nc.sync.dma_start(out=mask_sb[:], in_=maskadd[:])
            nc.sync.dma_start(out=mask01_sb[:], in_=mask01[:])
            nc.sync.dma_start(out=ones_bf[:], in_=ones_b[:])
            nc.sync.dma_start(out=ones_fr[64:65, :], in_=ones_f[:])
            nc.sync.dma_start(out=bq_sb[:], in_=bq_c[:])
            nc.sync.dma_start(out=bk_sb[:], in_=bk_c[:])
            nc.sync.dma_start(out=bv_sb[:], in_=bv_r[:])

            # ---- unified psum pool: "ps" 4 banks, "po" 3, "pb" 1 ----
            pp = tc.alloc_tile_pool(name="pp", bufs=3, space="PSUM")
            # ---- phase 1: projections ----
            with (
                tc.tile_pool(name="wbig", bufs=2) as wp,
                tc.tile_pool(name="xbig", bufs=3) as xp,
            ):
                # V first: out[s_tile, dv] = xvT_tile.T @ wvT (+ ones bias)
                wv_big = wp.tile([128, 8 * DG], BF16, tag="wb", name="wv_big")
                nc.gpsimd.dma_start(
                    out=wv_big[:].rearrange("p (k d) -> p k d", k=8),
                    in_=wv_t[:].rearrange("(k p) d -> p k d", p=128),
                )
                for s in range(16):
                    xv_big = xp.tile([128, 8 * 128], BF16, tag="xvb", name="xv_big")
                    nc.sync.dma_start(
                        out=xv_big[:].rearrange("p (k s2) -> p k s2", k=8),
                        in_=xv_t[:, s * 128 : (s + 1) * 128].rearrange(
                            "(k p) s2 -> p k s2", p=128
                        ),
                    )
                    ps = pp.tile([128, DG], F32, tag="ps", bufs=3, name="psv")
                    for k8 in range(8):
                        nc.tensor.matmul(
                            ps[:],
                            xv_big[:, k8 * 128 : (k8 + 1) * 128],
                            wv_big[:, k8 * DG : (k8 + 1) * DG],
                            start=(k8 == 0),
                            stop=False,
                        )
                    nc.tensor.matmul(
                        ps[:], ones_bf[:], bv_sb[:], start=False, stop=True
                    )
                    v3 = vt[s].rearrange("p (h x) -> p h x", x=65)
                    nc.vector.tensor_copy(
                        v3[:, :, 0:64],
                        ps[:].rearrange("p (h d) -> p h d", d=64),
                    )
                    nc.sync.dma_start(out=v3[:, :, 64:65], in_=ones8[:].unsqueeze(2))

                # Q^T and K^T interleaved so early attention heads unblock
                wq_big = wp.tile([128, 8 * DG], BF16, tag="wb", name="wq_big")
                wk_big = wp.tile([128, 8 * DG], BF16, tag="wb", name="wk_big")
                nc.gpsimd.dma_start(
                    out=wq_big[:].rearrange("p (k d) -> p k d", k=8),
                    in_=wq_t[:].rearrange("(k p) d -> p k d", p=128),
                )
                nc.gpsimd.dma_start(
                    out=wk_big[:].rearrange("p (k d) -> p k d", k=8),
                    in_=wk_t[:].rearrange("(k p) d -> p k d", p=128),
                )
                for n in range(4):
                    xq_big = xp.tile([128, 8 * 512], BF16, tag="xb", name="xq_big")
                    xk_big = xp.tile([128, 8 * 512], BF16, tag="xb", name="xk_big")
                    nc.sync.dma_start(
                        out=xq_big[:].rearrange("p (k s) -> p k s", k=8),
                        in_=xq_t[:, n * 512 : (n + 1) * 512].rearrange(
                            "(k p) s -> p k s", p=128
                        ),
                    )
                    nc.sync.dma_start(
                        out=xk_big[:].rearrange("p (k s) -> p k s", k=8),
                        in_=xk_t[:, n * 512 : (n + 1) * 512].rearrange(
                            "(k p) s -> p k s", p=128
                        ),
                    )
                    for m in range(4):
                        for w_big, x_big, b_sb, dst in (
                            (wq_big, xq_big, bq_sb, qT),
                            (wk_big, xk_big, bk_sb, kT),
                        ):
                            ps = pp.tile([128, 512], F32, tag="ps", bufs=3, name="ps1")
                            for k8 in range(8):
                                nc.tensor.matmul(
                                    ps[:],
                                    w_big[
                                        :,
                                        k8 * DG + m * 128 : k8 * DG + (m + 1) * 128,
                                    ],
                                    x_big[:, k8 * 512 : (k8 + 1) * 512],
                                    start=(k8 == 0),
                                    stop=(k8 == 7),
                                )
                            nc.scalar.activation(
                                dst[m][:, n * 512 : (n + 1) * 512],
                                ps[:],
                                AF.Identity,
                                bias=b_sb[:, m : m + 1],
                            )

            # ---- phase 2: causal attention (head pairs row-packed) ----
            with (
                tc.tile_pool(name="at", bufs=6) as ap_,
                tc.tile_pool(name="sm", bufs=8) as sm,
                tc.tile_pool(name="wo", bufs=4) as wop,
                tc.tile_pool(name="ob", bufs=4) as obp,
            ):
                wo_sb = [
                    wop.tile([128, D], BF16, tag="wo", name=f"wo{c}")
                    for c in range(4)
                ]
                for c in range(4):
                    nc.gpsimd.dma_start(
                        out=wo_sb[c][:], in_=wo_t[c * 128 : (c + 1) * 128, :]
                    )
                for p in range(4):
                    hA, hB = 2 * p, 2 * p + 1
                    for j in range(4):
                        nsk = 4 * j + 4
                        ps_oA = pp.tile([65, 512], F32, tag="po", bufs=3, name="ps_oA")
                        ps_oB = pp.tile([65, 512], F32, tag="po", bufs=3, name="ps_oB")
                        pending = None
                        for i in range(nsk):
                            koff = i - 4 * j
                            c0 = max(0, koff * 128) if koff >= 0 else 0
                            # two heads' scores row-packed: K=64 each at row
                            # groups 0 and 64 -> concurrent on the PE array
                            ps_sA = pp.tile([128, 512], F32, tag="ps", bufs=3, name="ps_sA")
                            ps_sB = pp.tile([128, 512], F32, tag="ps", bufs=3, name="ps_sB")
                            nc.tensor.matmul(
                                ps_sA[:, c0:512],
                                kT[p][0:64, i * 128 : (i + 1) * 128],
                                qT[p][0:64, j * 512 + c0 : (j + 1) * 512],
                                start=True,
                                stop=True,
                                tile_position=(0, 0),
                            )
                            nc.tensor.matmul(
                                ps_sB[:, c0:512],
                                kT[p][64:128, i * 128 : (i + 1) * 128],
                                qT[p][64:128, j * 512 + c0 : (j + 1) * 512],
                                start=True,
                                stop=True,
                                tile_position=(64, 0),
                            )
                            # software-pipeline skew: retire the PREVIOUS
                            # iteration's PVs while this iteration's exps run,
                            # so the PE never head-of-line blocks on ACT
                            if pending is not None:
                                pi, pc0, patA, patB = pending
                                nc.tensor.matmul(
                                    ps_oA[:, pc0:512],
                                    vt[pi][:, hA * 65 : hA * 65 + 65],
                                    patA[:, pc0:512],
                                    start=(pi == 0),
                                    stop=False,
                                )
                                nc.tensor.matmul(
                                    ps_oB[:, pc0:512],
                                    vt[pi][:, hB * 65 : hB * 65 + 65],
                                    patB[:, pc0:512],
                                    start=(pi == 0),
                                    stop=False,
                                )
                            atA = ap_.tile([128, 512], BF16, tag="at", name="atA")
                            atB = ap_.tile([128, 512], BF16, tag="at", name="atB")
                            nc.scalar.activation(
                                atA[:, c0:512], ps_sA[:, c0:512], AF.Exp, scale=0.125
                            )
                            nc.scalar.activation(
                                atB[:, c0:512], ps_sB[:, c0:512], AF.Exp, scale=0.125
                            )
                            if koff >= 0:
                                # zero the below-diagonal block post-exp (bf16)
                                nc.vector.tensor_mul(
                                    atA[:, c0 : c0 + 128],
                                    atA[:, c0 : c0 + 128],
                                    mask01_sb[:],
                                )
                                nc.vector.tensor_mul(
                                    atB[:, c0 : c0 + 128],
                                    atB[:, c0 : c0 + 128],
                                    mask01_sb[:],
                                )
                            pending = (i, c0, atA, atB)
                        pi, pc0, patA, patB = pending
                        nc.tensor.matmul(
                            ps_oA[:, pc0:512],
                            vt[pi][:, hA * 65 : hA * 65 + 65],
                            patA[:, pc0:512],
                            start=(pi == 0),
                            stop=True,
                        )
                        nc.tensor.matmul(
                            ps_oB[:, pc0:512],
                            vt[pi][:, hB * 65 : hB * 65 + 65],
                            patB[:, pc0:512],
                            start=(pi == 0),
                            stop=True,
                        )
                        # normalize both heads
                        for ps_o, hp in ((ps_oA, 0), (ps_oB, 64)):
                            den = sm.tile([65, 512], F32R, tag="den", name="den")
                            nc.vector.tensor_copy(den[64:65, :], ps_o[64:65, :])
                            ps_bc = pp.tile(
                                [64, 512], F32, tag="pw", bufs=2, name="ps_bc"
                            )
                            nc.tensor.matmul(
                                ps_bc[:],
                                ones_fr[64:65, 0:64],
                                den[64:65, :],
                                start=True,
                                stop=True,
                            )
                            rb = sm.tile([64, 512], F32, tag="rb", name="rb")
                            nc.vector.reciprocal(rb[:], ps_bc[:])
                            nc.vector.tensor_mul(ps_o[0:64, :], ps_o[0:64, :], rb[:])
                            dst = aout[p][hp : hp + 64, j * 512 : (j + 1) * 512]
                            if hp == 0:
                                nc.vector.tensor_copy(dst, ps_o[0:64, :])
                            else:
                                tmp = sm.tile([64, 512], BF16, tag="tmp", name="tmp")
                                nc.vector.tensor_copy(tmp[:], ps_o[0:64, :])
                                nc.sync.dma_start(out=dst, in_=tmp[:])
                    # dense full-array Wo burst for the completed pair
                    for s in range(16):
                        for n2 in range(2):
                            psw = pp.tile([128, 512], F32, tag="pw", bufs=2, name="psw")
                            nc.tensor.matmul(
                                psw[:],
                                aout[p][:, s * 128 : (s + 1) * 128],
                                wo_sb[p][:, n2 * 512 : (n2 + 1) * 512],
                                start=True,
                                stop=True,
                            )
                            ob = obp.tile([128, 512], F32, tag="ob", name="ob")
                            nc.vector.tensor_copy(ob[:], psw[:])
                            nc.gpsimd.dma_start(
                                out=outs[p][
                                    s * 128 : (s + 1) * 128,
                                    n2 * 512 : (n2 + 1) * 512,
                                ],
                                in_=ob[:],
                            )

            pp.release()

    nc.compile()
    return nc


def _make_in_maps(query, key, value, wq, bq, wk, bk, wv, bv, wo):
    f32 = np.float32
    ones_b = np.ones((1, 128), BF)
    ones_f = np.ones((1, 128), np.float32)
    ones8 = np.ones((128, 8), BF)
    # causal mask add-block in scores_T layout: rows=sk_local, cols=sq_local;
    # valid iff sq_local >= sk_local
    maskadd = np.where(
        np.triu(np.ones((128, 128), bool)), f32(0), f32(NEG)
    ).astype(f32)
    mask01 = np.triu(np.ones((128, 128), BF))

    wqT = np.asarray(wq, f32).T.astype(BF)  # [D, D] (d, dq)
    wkT = np.asarray(wk, f32).T.astype(BF)
    wvT = np.asarray(wv, f32).T.astype(BF)
    woT = np.asarray(wo, f32).T.astype(BF)  # [dv, D]

    in_maps = []
    for c in range(8):
        b, g = c // 2, c % 2
        sl = slice(g * DG, (g + 1) * DG)
        in_maps.append(
            {
                "xq_t": np.ascontiguousarray(np.asarray(query[b], f32).T.astype(BF)),
                "xk_t": np.ascontiguousarray(np.asarray(key[b], f32).T.astype(BF)),
                "xv_t": np.ascontiguousarray(np.asarray(value[b], f32).T.astype(BF)),
                "wq_t": np.ascontiguousarray(wqT[:, sl]),
                "wk_t": np.ascontiguousarray(wkT[:, sl]),
                "wv_t": np.ascontiguousarray(wvT[:, sl]),
                "wo_t": np.ascontiguousarray(woT[sl, :]),
                "bq_c": np.ascontiguousarray(
                    np.asarray(bq, f32)[sl].reshape(4, 128).T
                ),
                "bk_c": np.ascontiguousarray(
                    np.asarray(bk, f32)[sl].reshape(4, 128).T
                ),
                "bv_r": np.asarray(bv, f32)[sl].reshape(1, DG).astype(BF),
                "ones_b": ones_b,
                "ones_f": ones_f,
                "ones8": ones8,
                "maskadd": maskadd,
                "mask01": mask01,
            }
        )
    return in_maps


def kernel(query, key, value, mask, wq, bq, wk, bk, wv, bv, wo, bo):
    global _PROGRAM, LAST_RESULTS
    if _PROGRAM is None:
        _PROGRAM = _build_program()
    nc = _PROGRAM
    in_maps = _make_in_maps(query, key, value, wq, bq, wk, bk, wv, bv, wo)

    res = run_bass_kernel_spmd(nc, in_maps, core_ids=list(range(8)))
    LAST_RESULTS = res

    f32 = np.float32
    out = np.empty((B, S, D), f32)
    for b in range(B):
        acc = None
        for r in (res.results[2 * b], res.results[2 * b + 1]):
            for c in range(4):
                acc = r[f"out{c}"] if acc is None else acc + r[f"out{c}"]
        out[b] = acc
    out += np.asarray(bo, f32)[None, None, :]
    return out



# revision 5
# speedup vs baseline: 1.5928x; 1.5928x over previous
"""Multi-head attention (B=4, S=2048, D=1024, H=16, Dh=64) on 8 NeuronCores.

Sharding: core c handles batch b=c//2 and head-group g=c%2 (8 heads).
wq/wk/wv column-parallel, wo row-parallel; host sums the two partial
wo-products per batch and adds bo.

v2: engine-balanced software pipeline. The scalar (ACT) engine's exp
stream (~175us) and the PE's matmul stream (~200us) are co-critical, so
projections for pair p+1 are emitted as PE filler inside pair p's
attention, and the output projection is accumulated across all 4 pairs
in PSUM at the end (j-group staged against pair 3's attention). Exps
run 1024 wide over 2-bank PSUM score tiles to amortize ACT overhead.
Causal masking is a bf16 multiply on DVE with precomputed [128,1024]
patterns covering the diagonal double-blocks (including zeroing the
never-computed-psum regions that the wide exp sweeps over). Softmax
normalization: ones-column-augmented V gives the denominator on PSUM
row 64; reciprocal_approx_fast -> DMA hop to partition 0 -> gpsimd
partition_broadcast -> fused DVE multiply-cast into the bf16 attention
output (head B staged via SBUF->SBUF DMA for the partition shift).
"""

import sys

sys.path.insert(0, "/opt/trn_rl_repo")

import ml_dtypes
import numpy as np

import concourse.bass as bass  # noqa: F401
import concourse.bacc as bacc
import concourse.tile as tile
import concourse.mybir as mybir
from concourse.bass_utils import run_bass_kernel_spmd

F32 = mybir.dt.float32
BF16 = mybir.dt.bfloat16
AF = mybir.ActivationFunctionType
BF = ml_dtypes.bfloat16

B, S, D = 4, 2048, 1024
H, DH = 16, 64
HG = 8  # heads per core
DG = HG * DH  # 512 out-dims per core

_PROGRAM = None
LAST_RESULTS = None  # for test.py introspection


def _build_program():
    nc = bacc.Bacc("TRN2", target_bir_lowering=False, debug=False)

    xq_t = nc.dram_tensor("xq_t", [D, S], BF16, kind="ExternalInput")
    xk_t = nc.dram_tensor("xk_t", [D, S], BF16, kind="ExternalInput")
    xv_t = nc.dram_tensor("xv_t", [D, S], BF16, kind="ExternalInput")
    wq_t = nc.dram_tensor("wq_t", [D, DG], BF16, kind="ExternalInput")
    wk_t = nc.dram_tensor("wk_t", [D, DG], BF16, kind="ExternalInput")
    wv_t = nc.dram_tensor("wv_t", [D, DG], BF16, kind="ExternalInput")
    wo_t = nc.dram_tensor("wo_t", [DG, D], BF16, kind="ExternalInput")
    bq_c = nc.dram_tensor("bq_c", [128, 4], F32, kind="ExternalInput")
    bk_c = nc.dram_tensor("bk_c", [128, 4], F32, kind="ExternalInput")
    bv_r = nc.dram_tensor("bv_r", [1, DG], BF16, kind="ExternalInput")
    ones_b = nc.dram_tensor("ones_b", [1, 128], BF16, kind="ExternalInput")
    ones8 = nc.dram_tensor("ones8", [128, 8], BF16, kind="ExternalInput")
    m0t = nc.dram_tensor("m0t", [128, 1024], BF16, kind="ExternalInput")
    m1t = nc.dram_tensor("m1t", [128, 768], BF16, kind="ExternalInput")
    out_d = nc.dram_tensor("out", [S, D], F32, kind="ExternalOutput")

    with tile.TileContext(nc) as tc:
        with (
            nc.allow_low_precision(reason="bf16 attention pipeline"),
            tc.tile_pool(name="persist", bufs=1) as pers,
            tc.tile_pool(name="xb", bufs=3) as xp,
            tc.tile_pool(name="at", bufs=6) as ap_,
            tc.tile_pool(name="sm", bufs=2) as sm,
            tc.tile_pool(name="ob", bufs=3) as obp,
        ):
            # ---- persistent tiles ----
            qT = [pers.tile([128, S], BF16, name=f"qT{i}") for i in range(4)]
            kT = [pers.tile([128, S], BF16, name=f"kT{i}") for i in range(4)]
            # v tiles: [128 s, 8 heads x (64 v + 1 ones)]
            vt = [pers.tile([128, HG * 65], BF16, name=f"v{i}") for i in range(16)]
            aout = [pers.tile([128, S], BF16, name=f"ao{i}") for i in range(4)]
            wq_big = pers.tile([128, 8 * DG], BF16, name="wq_big")
            wk_big = pers.tile([128, 8 * DG], BF16, name="wk_big")
            wv_big = pers.tile([128, 8 * DG], BF16, name="wv_big")
            wo_sb = [pers.tile([128, D], BF16, name=f"wo{c}") for c in range(4)]
            m0 = pers.tile([128, 1024], BF16, name="m0")
            m1 = pers.tile([128, 768], BF16, name="m1")
            ones_bf = pers.tile([1, 128], BF16, name="ones_bf")
            bq_sb = pers.tile([128, 4], F32, name="bq")
            bk_sb = pers.tile([128, 4], F32, name="bk")
            bv_sb = pers.tile([1, DG], BF16, name="bv")

            nc.sync.dma_start(out=m0[:], in_=m0t[:])
            nc.sync.dma_start(out=m1[:], in_=m1t[:])
            nc.sync.dma_start(out=ones_bf[:], in_=ones_b[:])
            nc.sync.dma_start(out=bq_sb[:], in_=bq_c[:])
            nc.sync.dma_start(out=bk_sb[:], in_=bk_c[:])
            nc.sync.dma_start(out=bv_sb[:], in_=bv_r[:])
            nc.sync.dma_start(
                out=wq_big[:].rearrange("p (k d) -> p k d", k=8),
                in_=wq_t[:].rearrange("(k p) d -> p k d", p=128),
            )
            nc.sync.dma_start(
                out=wk_big[:].rearrange("p (k d) -> p k d", k=8),
                in_=wk_t[:].rearrange("(k p) d -> p k d", p=128),
            )
            nc.sync.dma_start(
                out=wv_big[:].rearrange("p (k d) -> p k d", k=8),
                in_=wv_t[:].rearrange("(k p) d -> p k d", p=128),
            )

            # PSUM: "s" scores 2x[128,1024] (4 banks), "o" attn-out 2x[65,512]
            # (2 banks), "p" proj/wo 2x[128,512] (2 banks)
            pp = tc.alloc_tile_pool(name="pp", bufs=2, space="PSUM")

            def proj_qk(m, n):
                """Project q,k for pair m, sequence chunk n (512 wide)."""
                xq_b = xp.tile([128, 8 * 512], BF16, tag="xqk", name="xq_b")
                xk_b = xp.tile([128, 8 * 512], BF16, tag="xqk", name="xk_b")
                nc.sync.dma_start(
                    out=xq_b[:].rearrange("p (k s) -> p k s", k=8),
                    in_=xq_t[:, n * 512 : (n + 1) * 512].rearrange(
                        "(k p) s -> p k s", p=128
                    ),
                )
                nc.sync.dma_start(
                    out=xk_b[:].rearrange("p (k s) -> p k s", k=8),
                    in_=xk_t[:, n * 512 : (n + 1) * 512].rearrange(
                        "(k p) s -> p k s", p=128
                    ),
                )
                for w_big, x_b, b_sb, dst in (
                    (wq_big, xq_b, bq_sb, qT),
                    (wk_big, xk_b, bk_sb, kT),
                ):
                    ps = pp.tile([128, 512], F32, tag="p", bufs=2, name="psp")
                    for k8 in range(8):
                        nc.tensor.matmul(
                            ps[:],
                            w_big[:, k8 * DG + m * 128 : k8 * DG + (m + 1) * 128],
                            x_b[:, k8 * 512 : (k8 + 1) * 512],
                            start=(k8 == 0),
                            stop=(k8 == 7),
                        )
                    nc.scalar.activation(
                        dst[m][:, n * 512 : (n + 1) * 512],
                        ps[:],
                        AF.Identity,
                        bias=b_sb[:, m : m + 1],
                    )

            def proj_v(s):
                """Project v for sequence tile s (128 rows)."""
                xv_b = xp.tile([128, 8 * 128], BF16, tag="xv", name="xv_b")
                nc.sync.dma_start(
                    out=xv_b[:].rearrange("p (k s2) -> p k s2", k=8),
                    in_=xv_t[:, s * 128 : (s + 1) * 128].rearrange(
                        "(k p) s2 -> p k s2", p=128
                    ),
                )
                ps = pp.tile([128, DG], F32, tag="p", bufs=2, name="psv")
                for k8 in range(8):
                    nc.tensor.matmul(
                        ps[:],
                        xv_b[:, k8 * 128 : (k8 + 1) * 128],
                        wv_big[:, k8 * DG : (k8 + 1) * DG],
                        start=(k8 == 0),
                        stop=False,
                    )
                nc.tensor.matmul(ps[:], ones_bf[:], bv_sb[:], start=False, stop=True)
                v3 = vt[s].rearrange("p (h x) -> p h x", x=65)
                nc.vector.tensor_copy(
                    v3[:, :, 0:64], ps[:].rearrange("p (h d) -> p h d", d=64)
                )
                nc.sync.dma_start(out=v3[:, :, 64:65], in_=ones8[:].unsqueeze(2))

            def attention(p, j):
                """Causal attention for head pair p, q chunk j (512 wide)."""
                hA, hB = 2 * p, 2 * p + 1
                ndblk = 2 * j + 2
                # custom-DVE reads of rotating PSUM slots resolve to the wrong
                # bank; keep ps_o at fixed banks via single-buffer tags
                ps_oA = pp.tile([65, 512], F32, tag="oA", bufs=1, name="ps_oA")
                ps_oB = pp.tile([65, 512], F32, tag="oB", bufs=1, name="ps_oB")
                for k in range(ndblk):
                    i0, i1 = 2 * k, 2 * k + 1
                    diag = i0 >= 4 * j
                    c0e = (i0 - 4 * j) * 128 if diag else 0
                    c0o = c0e + 128 if diag else 0
                    sA = pp.tile([128, 1024], F32, tag="s", bufs=2, name="sA")
                    sB = pp.tile([128, 1024], F32, tag="s", bufs=2, name="sB")
                    for hr, sX in ((0, sA), (64, sB)):
                        nc.tensor.matmul(
                            sX[:, c0e:512],
                            kT[p][hr : hr + 64, i0 * 128 : (i0 + 1) * 128],
                            qT[p][hr : hr + 64, j * 512 + c0e : (j + 1) * 512],
                            start=True,
                            stop=True,
                            tile_position=(hr, 0),
                        )
                        nc.tensor.matmul(
                            sX[:, 512 + c0o : 1024],
                            kT[p][hr : hr + 64, i1 * 128 : (i1 + 1) * 128],
                            qT[p][hr : hr + 64, j * 512 + c0o : (j + 1) * 512],
                            start=True,
                            stop=True,
                            tile_position=(hr, 0),
                        )
                    atA = ap_.tile([128, 1024], BF16, tag="at", name="atA")
                    atB = ap_.tile([128, 1024], BF16, tag="at", name="atB")
                    for sX, atX in ((sA, atA), (sB, atB)):
                        nc.scalar.activation(
                            atX[:, c0e:1024], sX[:, c0e:1024], AF.Exp, scale=0.125
                        )
                        if diag:
                            msk = m0 if c0e == 0 else m1
                            nc.vector.tensor_mul(
                                atX[:, c0e:1024], atX[:, c0e:1024], msk[:]
                            )
                    for h, ps_o, atX in ((hA, ps_oA, atA), (hB, ps_oB, atB)):
                        nc.tensor.matmul(
                            ps_o[:, c0e:512],
                            vt[i0][:, h * 65 : h * 65 + 65],
                            atX[:, c0e:512],
                            start=(i0 == 0),
                            stop=False,
                        )
                        nc.tensor.matmul(
                            ps_o[:, c0o:512],
                            vt[i1][:, h * 65 : h * 65 + 65],
                            atX[:, 512 + c0o : 1024],
                            start=False,
                            stop=(k == ndblk - 1),
                        )
                # normalize both heads: den on row 64 -> recip -> hop to
                # partition 0 -> gpsimd broadcast -> fused mul-cast
                for ps_o, hp in ((ps_oA, 0), (ps_oB, 64)):
                    # the custom-DVE op misresolves PSUM/partition-shifted
                    # APs; stage the denominator into a base-0 SBUF row with
                    # a standard copy first, then run all-SBUF base-0 ops
                    den_s = sm.tile([1, 512], F32, tag="dn", name="den_s")
                    nc.vector.tensor_copy(den_s[0:1, :], ps_o[64:65, :])
                    r0 = sm.tile([1, 512], F32, tag="r0", name="r0")
                    nc.vector.reciprocal_approx_fast(r0[0:1, :], den_s[0:1, :])
                    rb = sm.tile([64, 512], F32, tag="rb", name="rb")
                    nc.gpsimd.partition_broadcast(rb[:], r0[0:1, :])
                    dst = aout[p][hp : hp + 64, j * 512 : (j + 1) * 512]
                    if hp == 0:
                        nc.vector.tensor_mul(dst, ps_o[0:64, :], rb[:])
                    else:
                        tmp = sm.tile([64, 512], BF16, tag="tb", name="tmpB")
                        nc.vector.tensor_mul(tmp[:], ps_o[0:64, :], rb[:])
                        nc.sync.dma_start(out=dst, in_=tmp[:])

            def wo_group(j):
                """Output projection for sequence tiles 4j..4j+3, all pairs."""
                for s in range(4 * j, 4 * j + 4):
                    for n2 in range(2):
                        psw = pp.tile([128, 512], F32, tag="p", bufs=2, name="psw")
                        for c in range(4):
                            nc.tensor.matmul(
                                psw[:],
                                aout[c][:, s * 128 : (s + 1) * 128],
                                wo_sb[c][:, n2 * 512 : (n2 + 1) * 512],
                                start=(c == 0),
                                stop=(c == 3),
                            )
                        ob = obp.tile([128, 512], F32, tag="ob", name="ob")
                        nc.vector.tensor_copy(ob[:], psw[:])
                        nc.gpsimd.dma_start(
                            out=out_d[
                                s * 128 : (s + 1) * 128, n2 * 512 : (n2 + 1) * 512
                            ],
                            in_=ob[:],
                        )

            # ---- emission order = scheduler priority ----
            proj_qk(0, 0)
            for s in range(4):
                proj_v(s)
            attention(0, 0)
            for jj in range(1, 4):
                proj_qk(0, jj)
                for s in range(4 * jj, 4 * jj + 4):
                    proj_v(s)
                attention(0, jj)
            for n in range(4):
                proj_qk(1, n)
            for jj in range(4):
                attention(1, jj)
            for n in range(4):
                proj_qk(2, n)
            for n in range(4):
                proj_qk(3, n)
            for jj in range(4):
                attention(2, jj)
            for c in range(4):
                nc.sync.dma_start(
                    out=wo_sb[c][:], in_=wo_t[c * 128 : (c + 1) * 128, :]
                )
            for jj in range(4):
                attention(3, jj)
                wo_group(jj)

            pp.release()

    nc.compile()
    return nc


def _make_masks():
    f1 = np.ones
    tri = np.triu(np.ones((128, 128), np.float32))  # 1 iff col >= row
    z = np.zeros
    m0 = np.concatenate(
        [tri, f1((128, 384), np.float32), z((128, 128), np.float32), tri,
         f1((128, 256), np.float32)],
        axis=1,
    ).astype(BF)
    m1 = np.concatenate(
        [tri, f1((128, 128), np.float32), z((128, 384), np.float32), tri],
        axis=1,
    ).astype(BF)
    return np.ascontiguousarray(m0), np.ascontiguousarray(m1)


def _make_in_maps(query, key, value, wq, bq, wk, bk, wv, bv, wo):
    f32 = np.float32
    ones_b = np.ones((1, 128), BF)
    ones8 = np.ones((128, 8), BF)
    m0, m1 = _make_masks()

    wqT = np.asarray(wq, f32).T.astype(BF)  # [D, D] (d, dq)
    wkT = np.asarray(wk, f32).T.astype(BF)
    wvT = np.asarray(wv, f32).T.astype(BF)
    woT = np.asarray(wo, f32).T.astype(BF)  # [dv, D]

    in_maps = []
    for c in range(8):
        b, g = c // 2, c % 2
        sl = slice(g * DG, (g + 1) * DG)
        in_maps.append(
            {
                "xq_t": np.ascontiguousarray(np.asarray(query[b], f32).T.astype(BF)),
                "xk_t": np.ascontiguousarray(np.asarray(key[b], f32).T.astype(BF)),
                "xv_t": np.ascontiguousarray(np.asarray(value[b], f32).T.astype(BF)),
                "wq_t": np.ascontiguousarray(wqT[:, sl]),
                "wk_t": np.ascontiguousarray(wkT[:, sl]),
                "wv_t": np.ascontiguousarray(wvT[:, sl]),
                "wo_t": np.ascontiguousarray(woT[sl, :]),
                "bq_c": np.ascontiguousarray(
                    np.asarray(bq, f32)[sl].reshape(4, 128).T
                ),
                "bk_c": np.ascontiguousarray(
                    np.asarray(bk, f32)[sl].reshape(4, 128).T
                ),
                "bv_r": np.asarray(bv, f32)[sl].reshape(1, DG).astype(BF),
                "ones_b": ones_b,
                "ones8": ones8,
                "m0t": m0,
                "m1t": m1,
            }
        )
    return in_maps


def kernel(query, key, value, mask, wq, bq, wk, bk, wv, bv, wo, bo):
    global _PROGRAM, LAST_RESULTS
    if _PROGRAM is None:
        _PROGRAM = _build_program()
    nc = _PROGRAM
    in_maps = _make_in_maps(query, key, value, wq, bq, wk, bk, wv, bv, wo)

    res = run_bass_kernel_spmd(nc, in_maps, core_ids=list(range(8)))
    LAST_RESULTS = res

    f32 = np.float32
    out = np.empty((B, S, D), f32)
    for b in range(B):
        out[b] = res.results[2 * b]["out"] + res.results[2 * b + 1]["out"]
    out += np.asarray(bo, f32)[None, None, :]
    return out
